# revision 2
# baseline (speedup 1.0000x reference)
"""Multi-head attention (B=4, S=1024, H=1024, 16 heads) on 8 trn2 cores.

E6: pair streams iterate n-half (phase) outer, sk inner. Per slot the two
heads' logits matmuls are row-packed (K=64, row groups 0-63 / 64-127, run
concurrently on the PE) into the two banks of ONE [128,1024] psum tile, so
a single wide exp covers both heads — halving ACT instruction overhead.
attn@V accumulates per (head, n-half) in [65,512] psum tiles (ones column
-> softmax denominator in row 64). Projection/output-projection matmuls
interleave into the PE slack of the ACT-paced stream. Input slab pools are
hoisted out of the unroll loop with xt/yt/wv double-buffered so the next
iteration's input DMAs overlap compute.

Sharding: 8 shards = (batch b in 0..3) x (head-half hf in 0..1), host sums
the two row-parallel Wo partials per batch.

PSUM (8 banks): av 2x[128,512]=2, lg 2x[128,1024]=4, sc 2.
"""

import numpy as np
import ml_dtypes

import concourse.bass as bass
import concourse.tile as tile
from concourse import bacc, mybir
from concourse import bass_utils

F32 = mybir.dt.float32
BF16 = mybir.dt.bfloat16
EXP = mybir.ActivationFunctionType.Exp
COPY = mybir.ActivationFunctionType.Copy

S = 1024  # sequence length (tokens)
HID = 1024  # model hidden
DQ = 512  # per-core projected dim (8 heads x 64)
NHL = 8  # local heads per core
DH = 64  # head depth
NK = HID // 128  # 8 contraction tiles over hidden
P = 128
N_CORES = 8

_CACHED_NC = None


def build_program(unroll=1):
    nc = bacc.Bacc("TRN2", target_bir_lowering=False, debug=False)
    xt = nc.dram_tensor("xt", [P, 2 * NK * 512], BF16, kind="ExternalInput").ap()
    yt = nc.dram_tensor("yt", [P, 2 * NK * 512], BF16, kind="ExternalInput").ap()
    wq = nc.dram_tensor("wq", [P, 4 * NK * P], BF16, kind="ExternalInput").ap()
    wk = nc.dram_tensor("wk", [P, 4 * NK * P], BF16, kind="ExternalInput").ap()
    wv = nc.dram_tensor("wv", [P, NK * DQ], BF16, kind="ExternalInput").ap()
    wo = nc.dram_tensor("wo", [P, 4 * HID], BF16, kind="ExternalInput").ap()
    biasd = nc.dram_tensor("biasd", [P, NK], F32, kind="ExternalInput").ap()
    onesd = nc.dram_tensor("onesd", [P, NHL], BF16, kind="ExternalInput").ap()
    identd = nc.dram_tensor("identd", [P, P], BF16, kind="ExternalInput").ap()
    out = nc.dram_tensor("out", [S, HID], BF16, kind="ExternalOutput").ap()

    with tile.TileContext(nc) as tc:
        with (
            tc.tile_pool(name="inpool2", bufs=2) as inpool2,
            tc.tile_pool(name="inpool1", bufs=1) as inpool1,
            tc.tile_pool(name="qkv", bufs=1) as qkvpool,
            tc.tile_pool(name="atp", bufs=1) as atpool,
            tc.tile_pool(name="expp", bufs=8) as exppool,
            tc.tile_pool(name="smallp", bufs=4) as smallpool,
            tc.tile_pool(name="accp", bufs=1) as accpool,
            tc.tile_pool(name="outp", bufs=4) as outpool,
        ):
            pools = (inpool2, inpool1, qkvpool, atpool, exppool, smallpool,
                     accpool, outpool)
            for _ in range(unroll):
                emit_kernel(tc, pools, out, xt, yt, wq, wk, wv, wo, biasd,
                            onesd, identd)
    nc.compile()
    return nc


def emit_kernel(tc, pools, out, xt, yt, wq, wk, wv, wo, biasd, onesd, identd):
    nc = tc.nc
    (inpool2, inpool1, qkvpool, atpool, exppool, smallpool, accpool,
     outpool) = pools
    if True:
        # ---- input slabs (DMA'd in large consolidated transfers);
        # xt/yt/wv double-buffered across unroll iterations ----
        wv_slab = inpool2.tile([P, NK * DQ], BF16, tag="wv", name="wv_slab")
        yt_slab = inpool2.tile([P, NK * S], BF16, tag="yt", name="yt_slab")
        xt_slab = inpool2.tile([P, NK * S], BF16, tag="xt", name="xt_slab")
        wq_slab = inpool1.tile([P, 4 * NK * P], BF16, tag="wq", name="wq_slab")
        wk_slab = inpool1.tile([P, 4 * NK * P], BF16, tag="wk", name="wk_slab")
        wo_slab = inpool1.tile([P, 4 * HID], BF16, tag="wo", name="wo_slab")
        bias_sb = inpool1.tile([P, NK], F32, tag="bias", name="bias_sb")
        vones_sb = inpool1.tile([P, NHL], BF16, tag="vones", name="vones_sb")
        ident_sb = inpool1.tile([P, P], BF16, tag="ident", name="ident_sb")

        # issue order on SP = earliest-needed first
        yt3 = yt_slab[:].rearrange("p (k c) -> p k c", c=S)
        xt3 = xt_slab[:].rearrange("p (k c) -> p k c", c=S)
        for q in range(8):
            if q >= 2 and q % 2 == 1:
                continue  # eighth-split only the first quarter
            span = 1 if q < 2 else 2
            wvs = slice(q * (NK * DQ // 8), (q + span) * (NK * DQ // 8))
            nc.sync.dma_start(wv_slab[:, wvs], wv[:, wvs])
            nc.sync.dma_start(
                yt3[:, q : q + span, 0:512],
                yt[:, q * (NK * 512 // 8) : (q + span) * (NK * 512 // 8)],
            )
        sl0 = slice(0, NK * P)
        nc.sync.dma_start(wk_slab[:, sl0], wk[:, sl0])
        nc.sync.dma_start(yt3[:, :, 512:1024], yt[:, NK * 512 : 2 * NK * 512])
        nc.sync.dma_start(bias_sb[:], biasd[:])
        nc.sync.dma_start(vones_sb[:], onesd[:])
        nc.sync.dma_start(ident_sb[:], identd[:])
        nc.sync.dma_start(wq_slab[:, sl0], wq[:, sl0])
        nc.sync.dma_start(xt3[:, :, 0:512], xt[:, 0 : NK * 512])
        nc.sync.dma_start(xt3[:, :, 512:1024], xt[:, NK * 512 : 2 * NK * 512])
        for pair in range(1, 4):
            sl = slice(pair * NK * P, (pair + 1) * NK * P)
            nc.sync.dma_start(wq_slab[:, sl], wq[:, sl])
            nc.sync.dma_start(wk_slab[:, sl], wk[:, sl])
        for pair in range(4):
            sl = slice(pair * HID, (pair + 1) * HID)
            nc.sync.dma_start(wo_slab[:, sl], wo[:, sl])

        def wv_k(k):
            return wv_slab[:, k * DQ : (k + 1) * DQ]

        def yt_k(k):
            return yt_slab[:, k * S : (k + 1) * S]

        def xt_k(k):
            return xt_slab[:, k * S : (k + 1) * S]

        def wqk_pk(slab, pair, k):
            base = pair * NK * P + k * P
            return slab[:, base : base + P]

        def wo_p(pair):
            return wo_slab[:, pair * HID : (pair + 1) * HID]

        # ---- persistent slabs ----
        qt_sb = [
            qkvpool.tile([P, S], BF16, tag=f"qt{m}", name=f"qt{m}") for m in range(4)
        ]
        kt_sb = [
            qkvpool.tile([P, S], BF16, tag=f"kt{m}", name=f"kt{m}") for m in range(4)
        ]
        v_sb = [
            qkvpool.tile([P, NHL * 2 * DH], BF16, tag=f"v{m}", name=f"v{m}")
            for m in range(8)
        ]
        at_sb = [
            atpool.tile([P, S], BF16, tag=f"at{m}", name=f"at{m}") for m in range(4)
        ]
        acc_sb = [
            accpool.tile([P, HID], BF16, tag=f"acc{m}", name=f"acc{m}")
            for m in range(8)
        ]

        # PSUM (8 banks): av 2x[65,512]=2, lg 2x[128,1024]=4, sc 2
        pp_av = tc.alloc_tile_pool(name="pp_av", bufs=2, space="PSUM")
        pp_lg = tc.alloc_tile_pool(name="pp_lg", bufs=2, space="PSUM")
        pp_sc = tc.alloc_tile_pool(name="pp_sc", bufs=2, space="PSUM")

        # ---- V projection (token-major, ones columns appended) ----
        def v_pair(mp):
            pss = {}
            for kk in range(2):
                for m in (mp, mp + 1):
                    if kk == 0:
                        pss[m] = pp_sc.tile([P, DQ], F32, tag="sc", name="sc")
                    for k in range(kk * NK // 2, (kk + 1) * NK // 2):
                        nc.tensor.matmul(
                            pss[m][:],
                            yt_k(k)[:, m * P : (m + 1) * P],
                            wv_k(k),
                            start=(k == 0),
                            stop=(k == NK - 1),
                        )
            for m in (mp, mp + 1):
                dst3 = v_sb[m][:].rearrange("p (h c) -> p h c", c=2 * DH)
                src3 = pss[m][:].rearrange("p (h c) -> p h c", c=DH)
                nc.vector.tensor_copy(dst3[:, :, 0:DH], src3[:, :, :])
                # ones block: av matmul rows 64-127 become the replicated
                # softmax denominator
                nc.vector.memset(dst3[:, :, DH : 2 * DH], 1.0)

        # ---- QT/KT projection for one pair as 32 emit-chunks of 1 matmul ----
        def proj_chunks(pair):
            chunks = []
            for w_slab, src_k, dst in (
                (wq_slab, xt_k, qt_sb),
                (wk_slab, yt_k, kt_sb),
            ):
                for n in range(2):
                    ps_box = [None]

                    def mm(k, w_slab=w_slab, src_k=src_k, dst=dst, n=n, ps_box=ps_box):
                        if k == 0:
                            ps_box[0] = pp_sc.tile([P, 512], F32, tag="sc", name="sc")
                        nc.tensor.matmul(
                            ps_box[0][:],
                            wqk_pk(w_slab, pair, k),
                            src_k(k)[:, n * 512 : (n + 1) * 512],
                            start=(k == 0),
                            stop=(k == NK - 1),
                        )
                        if k == NK - 1:
                            nc.vector.tensor_copy(
                                dst[pair][:, n * 512 : (n + 1) * 512], ps_box[0][:]
                            )

                    for k in range(NK):
                        chunks.append(lambda k=k, mm=mm: mm(k))
            return chunks

        # ---- pairs 0-2 of the output projection ----
        def wo012_chunks():
            chunks = []
            for m in range(8):
                for n in range(2):
                    ps_box = [None]
                    act_evict = 2 * m + n >= 16 - WO_RESERVE

                    def part1(m=m, n=n, ps_box=ps_box):
                        ps_box[0] = pp_sc.tile([P, 512], F32, tag="sc", name="sc")
                        for pair in range(2):
                            nc.tensor.matmul(
                                ps_box[0][:],
                                at_sb[pair][:, m * P : (m + 1) * P],
                                wo_p(pair)[:, n * 512 : (n + 1) * 512],
                                start=(pair == 0),
                                stop=False,
                            )

                    def part2(m=m, n=n, ps_box=ps_box, act_evict=act_evict):
                        nc.tensor.matmul(
                            ps_box[0][:],
                            at_sb[2][:, m * P : (m + 1) * P],
                            wo_p(2)[:, n * 512 : (n + 1) * 512],
                            start=False,
                            stop=True,
                        )
                        dst = acc_sb[m][:, n * 512 : (n + 1) * 512]
                        if act_evict:
                            nc.scalar.activation(dst, ps_box[0][:], COPY)
                        else:
                            nc.vector.tensor_copy(dst, ps_box[0][:])

                    chunks.append(part1)
                    chunks.append(part2)
            return chunks

        # ---- pair section: two heads per stream; logits row-packed (K=64
        # row groups 0-63 / 64-127 run concurrently on the PE), exp for both
        # heads back-to-back on ACT, attn@V lagged via the deferred queue ----
        LAG = 3
        WO_RESERVE = 2
        pending = {}
        gctr = [0]

        def emit_pair(pair, extras, delay=0):
            hA, hB = 2 * pair, 2 * pair + 1
            ei = 0
            nslots = 2 * NK
            for j in range(nslots):
                g = gctr[0]
                gctr[0] += 1
                n, sk = divmod(j, NK)
                if sk == 0:
                    # per (head, n-half) accumulators; rows 64-127 hold the
                    # replicated softmax denominator (64 ones-columns in v)
                    avs = {
                        hA: pp_av.tile([P, 512], F32, tag="av", name="av"),
                        hB: pp_av.tile([P, 512], F32, tag="av", name="av"),
                    }
                # row-packed logits: head A (rows 0-63) -> left bank, head B
                # (rows 64-127) -> right bank of one [128,1024] psum tile
                lg = pp_lg.tile([P, 2 * 512], F32, tag="lg", name="lg")
                for h, base in ((hA, 0), (hB, DH)):
                    nc.tensor.matmul(
                        lg[:, base * 8 : base * 8 + 512],
                        kt_sb[pair][base : base + DH, sk * P : (sk + 1) * P],
                        qt_sb[pair][base : base + DH, n * 512 : (n + 1) * 512],
                        start=True,
                        stop=True,
                    )
                e = exppool.tile([P, 2 * 512], BF16, tag="exp", name="exp")
                nc.scalar.activation(
                    e[:], lg[:], EXP, bias=bias_sb[:, sk : sk + 1]
                )
                for fn in pending.pop(g, []):
                    fn()
                if j >= delay:
                    take = (len(extras) - ei + (nslots - 1 - j)) // (
                        nslots - j - (delay - j if j < delay else 0)
                    )
                    for _ in range(take):
                        extras[ei]()
                        ei += 1

                def av_mm(sk=sk, e=e, avs=avs, hA=hA, hB=hB):
                    for li, h in enumerate((hA, hB)):
                        nc.tensor.matmul(
                            avs[h][:],
                            v_sb[sk][:, h * 2 * DH : (h + 1) * 2 * DH],
                            e[:, li * 512 : (li + 1) * 512],
                            start=(sk == 0),
                            stop=(sk == NK - 1),
                        )

                pending.setdefault(g + LAG, []).append(av_mm)
                if sk == NK - 1:

                    def norm(h, n, avs=avs, pair=pair):
                        av = avs[h]
                        base = (h % 2) * DH
                        cs = slice(n * 512, (n + 1) * 512)
                        rc = smallpool.tile([DH, 512], F32, tag="rc", name="rc")
                        nc.vector.reciprocal(rc[:], av[DH : 2 * DH, :])
                        nc.vector.tensor_mul(
                            at_sb[pair][base : base + DH, cs],
                            av[0:DH, :],
                            rc[:],
                        )

                    for h in (hA, hB):
                        pending.setdefault(g + LAG, []).append(
                            lambda norm=norm, h=h, n=n: norm(h=h, n=n)
                        )
            assert ei == len(extras)

        # startup: V and pair-0 projections interleaved by data arrival
        p0 = proj_chunks(0)
        v_pair(0)
        v_pair(2)
        for ch in p0[16:24]:  # K proj, n=0 half
            ch()
        v_pair(4)
        v_pair(6)
        for ch in p0[24:32]:  # K proj, n=1 half
            ch()
        for ch in p0[0:16]:  # Q proj
            ch()
        # pairs 0-2 carry the next pair's projections; pair 3 carries the
        # pair-0..2 output projection groups
        for pair in range(3):
            emit_pair(pair, proj_chunks(pair + 1))
        wo012 = wo012_chunks()
        nres = 2 * WO_RESERVE
        emit_pair(3, wo012[: 32 - nres], delay=3)
        # flush trailing lagged attn@V + normalize closures, interleaving the
        # reserved wo012 chunks so the PE stays busy through the normalize
        reserved = wo012[32 - nres :]
        flush = []
        for g in sorted(pending.keys()):
            flush.extend(pending.pop(g))
        fi = ri = 0
        while fi < len(flush) or ri < len(reserved):
            if fi < len(flush):
                flush[fi]()
                fi += 1
            if ri < len(reserved):
                reserved[ri]()
                ri += 1

        # ---- tail: per m-tile PSUM group = identity matmul injecting the
        # bf16 pairs-0..2 partial + the pair-3 matmul; one ACT/DVE copy ->
        # bf16 streams out ----
        pp_sc.release()
        pp_lg.release()
        pp_tail = tc.alloc_tile_pool(name="pp_tail", bufs=4, space="PSUM")
        WARM = 4
        units = [(m, n) for m in range(8) for n in range(2)]
        tail_ps = {}
        ob_tiles = {}

        def emit_ident(u):
            m, n = units[u]
            ps = pp_tail.tile([P, 512], F32, tag="tl", name="tl")
            tail_ps[u] = ps
            nc.tensor.matmul(
                ps[:],
                ident_sb[:],
                acc_sb[m][:, n * 512 : (n + 1) * 512],
                start=True,
                stop=False,
            )

        for u in range(WARM):
            emit_ident(u)
        for u in range(16):
            m, n = units[u]
            ps = tail_ps[u]
            nc.tensor.matmul(
                ps[:],
                at_sb[3][:, m * P : (m + 1) * P],
                wo_p(3)[:, n * 512 : (n + 1) * 512],
                start=False,
                stop=True,
            )
            if u + WARM < 16:
                emit_ident(u + WARM)
            if n == 0:
                ob_tiles[m] = outpool.tile([P, HID], BF16, tag="ob", name="ob")
            ob = ob_tiles[m]
            dst = ob[:, n * 512 : (n + 1) * 512]
            if u % 2 == 1:
                nc.vector.tensor_copy(dst, ps[:])
            else:
                nc.scalar.activation(dst, ps[:], COPY)
            if n == 1:
                nc.sync.dma_start(out[m * P : (m + 1) * P, :], ob[:])
        pp_tail.release()
        pp_av.release()


def _prep_in_maps(x, y, bias, Wq, Wk, Wv, Wo):
    x = np.asarray(x, dtype=np.float32)
    y = np.asarray(y, dtype=np.float32)
    bias = np.asarray(bias, dtype=np.float32)
    Wq = np.asarray(Wq, dtype=np.float32)
    Wk = np.asarray(Wk, dtype=np.float32)
    Wv = np.asarray(Wv, dtype=np.float32)
    Wo = np.asarray(Wo, dtype=np.float32)
    scale = 1.0 / np.sqrt(DH)
    dt = ml_dtypes.bfloat16

    def act_slab(a):
        at = a.T.reshape(NK, P, 2, 512)  # [k, p, h, c]
        return np.ascontiguousarray(at.transpose(1, 2, 0, 3).reshape(P, 2 * NK * 512))

    def w_pair_slab(w):
        wr = w.reshape(NK, P, 4, P)  # [k, p, pair, c]
        return np.ascontiguousarray(wr.transpose(1, 2, 0, 3).reshape(P, 4 * NK * P))

    def wv_slab(w):
        wr = w.reshape(NK, P, DQ)
        return np.ascontiguousarray(wr.transpose(1, 0, 2).reshape(P, NK * DQ))

    def wo_slab(w):
        wr = w.reshape(4, P, HID)
        return np.ascontiguousarray(wr.transpose(1, 0, 2).reshape(P, 4 * HID))

    in_maps = []
    for c in range(N_CORES):
        b, hf = divmod(c, 2)
        cols = slice(hf * DQ, (hf + 1) * DQ)
        in_maps.append(
            {
                "xt": act_slab(x[b]).astype(dt),
                "yt": act_slab(y[b]).astype(dt),
                "wq": w_pair_slab(Wq[:, cols] * scale).astype(dt),
                "wk": w_pair_slab(Wk[:, cols]).astype(dt),
                "wv": wv_slab(Wv[:, cols]).astype(dt),
                "wo": wo_slab(Wo[cols, :]).astype(dt),
                "biasd": np.ascontiguousarray(bias[b, 0, 0].reshape(NK, P).T),
                "onesd": np.ones((P, NHL), dtype=dt),
                "identd": np.eye(P, dtype=dt),
            }
        )
    return in_maps


def get_program():
    global _CACHED_NC
    if _CACHED_NC is None:
        _CACHED_NC = build_program()
    return _CACHED_NC


def kernel(x, y, bias, Wq, Wk, Wv, Wo):
    nc = get_program()
    in_maps = _prep_in_maps(x, y, bias, Wq, Wk, Wv, Wo)
    res = bass_utils.run_bass_kernel_spmd(nc, in_maps, core_ids=list(range(N_CORES)))
    B = 4
    out = np.empty((B, S, HID), dtype=np.float32)
    for b in range(B):
        out[b] = res.results[2 * b]["out"].astype(np.float32) + res.results[2 * b + 1][
            "out"
        ].astype(np.float32)
    return out
